# revision 34
# baseline (speedup 1.0000x reference)
from contextlib import ExitStack

import numpy as np
import ml_dtypes

import concourse.bass as bass
import concourse.mybir as mybir
import concourse.tile as tile
from concourse import bacc
from concourse.bass_utils import run_bass_kernel_spmd

B, N, C, H, D = 4, 2048, 256, 4, 64
NCORES = 8
Q = 1024
SCALE = float(D) ** -0.5
FP32 = mybir.dt.float32
BF16 = mybir.dt.bfloat16
FP8E4 = mybir.dt.float8e4
FP8E5 = mybir.dt.float8e5
U8 = mybir.dt.uint8
AF = mybir.ActivationFunctionType
DR = mybir.MatmulPerfMode.DoubleRow
ALU = mybir.AluOpType

A5 = 4.0 * np.log2(np.e) * SCALE
B5 = 60.0


def dve_tile(g, j):
    return ((2 * g + j) % 3 == 1 or g % 8 == 6) and g % 8 < 7


_CACHE = {}
FINE_P1 = False
P1_SPLIT = False
POP_FROM = 1
NORM_KC = 3
EVAC_BOTH_SCALAR = False


def build_nc(dve_rule=None, warmup_n=10):
    if dve_rule is None:
        dve_rule = dve_tile
    nc = bacc.Bacc("TRN2", target_bir_lowering=False, debug=False,
                   num_devices=NCORES)

    t1T_d = nc.dram_tensor("t1T", [C, N], BF16, kind="ExternalInput")
    t2T_d = nc.dram_tensor("t2T", [C, Q], BF16, kind="ExternalInput")
    wq_d = nc.dram_tensor("wq", [C, C], BF16, kind="ExternalInput")
    wk_d = nc.dram_tensor("wk", [C, C], BF16, kind="ExternalInput")
    wv_d = nc.dram_tensor("wv", [C, C], BF16, kind="ExternalInput")
    wc_d = nc.dram_tensor("wc", [C, C], BF16, kind="ExternalInput")
    wph_d = nc.dram_tensor("wph", [4 * 65, C], BF16, kind="ExternalInput")
    out_d = nc.dram_tensor("out", [C, Q], BF16, kind="ExternalOutput")

    with tile.TileContext(nc) as tc, ExitStack() as ctx:
        const = ctx.enter_context(tc.tile_pool(name="const", bufs=1))
        acts = ctx.enter_context(tc.tile_pool(name="acts", bufs=1))

        w_sb = {n: [const.tile([128, C], BF16, name=f"{n}{cc}", tag=f"{n}{cc}")
                    for cc in range(2)]
                for n in ("wk", "wq", "wv", "wc")}
        wph = [const.tile([65, C], BF16, name=f"wph{h}", tag=f"wph{h}")
               for h in range(4)]
        t1T = [acts.tile([128, N], BF16, name=f"t1T{cc}", tag=f"t1T{cc}")
               for cc in range(2)]
        t2T = [acts.tile([128, Q], BF16, name=f"t2T{cc}", tag=f"t2T{cc}")
               for cc in range(2)]

        kT = [acts.tile([128, N], BF16, name=f"kT{m}", tag=f"kT{m}")
              for m in range(2)]
        qT = [acts.tile([128, Q], BF16, name=f"qT{m}", tag=f"qT{m}")
              for m in range(2)]
        wmat = const.tile([128, 512], BF16, name="wmat", tag="wmat")
        nc.gpsimd.memset(wmat[:], 0.0)
        v_sb = acts.tile([128, 8 * 640], FP8E4, name="v", tag="v")
        for kcp in range(8):
            nc.gpsimd.memset(
                v_sb[:, kcp * 640:(kcp + 1) * 640]
                .rearrange("p (t h c) -> p t h c", t=2, h=4)[:, :, :, 0:1],
                1.0)
        xon = [acts.tile([65, Q], BF16, name=f"xon{h}", tag=f"xon{h}")
               for h in range(4)]

        pes_sb = acts.tile([128, 16 * 2048], FP8E5, name="pes", tag="pes")
        npool = ctx.enter_context(tc.tile_pool(name="norm", bufs=2))
        osb = ctx.enter_context(tc.tile_pool(name="osb", bufs=2))

        spool = ctx.enter_context(
            tc.tile_pool(name="spsum", bufs=3, space="PSUM"))
        xopool = ctx.enter_context(
            tc.tile_pool(name="xopsum", bufs=1, space="PSUM"))

        qs = (nc.sync, nc.scalar)
        for cc in range(2):
            qs[cc].dma_start(out=w_sb["wk"][cc][:],
                             in_=wk_d[cc * 128:(cc + 1) * 128, :])
        for cc in range(2):
            qs[cc].dma_start(out=t1T[cc][:, 0:512],
                             in_=t1T_d[cc * 128:(cc + 1) * 128, 0:512])
        for cc in range(2):
            qs[cc].dma_start(out=w_sb["wq"][cc][:],
                             in_=wq_d[cc * 128:(cc + 1) * 128, :])
        for cc in range(2):
            qs[cc].dma_start(out=t2T[cc][:, 0:512],
                             in_=t2T_d[cc * 128:(cc + 1) * 128, 0:512])
        for cc in range(2):
            qs[cc].dma_start(out=t1T[cc][:, 512:1024],
                             in_=t1T_d[cc * 128:(cc + 1) * 128, 512:1024])
        for cc in range(2):
            qs[cc].dma_start(out=w_sb["wv"][cc][:],
                             in_=wv_d[cc * 128:(cc + 1) * 128, :])
        for cc in range(2):
            qs[cc].dma_start(out=t2T[cc][:, 512:1024],
                             in_=t2T_d[cc * 128:(cc + 1) * 128, 512:1024])
        for cc in range(2):
            qs[cc].dma_start(out=t1T[cc][:, 1024:2048],
                             in_=t1T_d[cc * 128:(cc + 1) * 128, 1024:2048])

        warm = const.tile([1, 16], FP32, name="warm", tag="warm")
        nc.gpsimd.memset(warm[:], 0.0)
        nc.scalar.activation(warm[:], warm[:], AF.Exp)
        wxo = xopool.tile([65, 512], FP32, name="xo0", tag="xo0")
        for _ in range(warmup_n):
            nc.tensor.matmul(wxo[0:65, :], lhsT=wmat[:, 0:65], rhs=wmat[:],
                             start=True, stop=True, skip_group_check=True)

        def emit_kT(m, blk, halves=(0, 1)):
            ps = spool.tile([128, 1024], FP32, name="s", tag="s")
            for j in halves:
                for cc in range(2):
                    nc.tensor.matmul(
                        ps[:, j * 512:(j + 1) * 512],
                        lhsT=w_sb["wk"][cc][:, m * 128:(m + 1) * 128],
                        rhs=t1T[cc][:, blk * 1024 + j * 512:
                                    blk * 1024 + (j + 1) * 512],
                        start=(cc == 0), stop=(cc == 1))
                nc.vector.tensor_copy(
                    kT[m][:, blk * 1024 + j * 512:blk * 1024 + (j + 1) * 512],
                    ps[:, j * 512:(j + 1) * 512])

        def emit_qT(m, halves=(0, 1)):
            ps = spool.tile([128, 1024], FP32, name="s", tag="s")
            for j in halves:
                for cc in range(2):
                    nc.tensor.matmul(
                        ps[:, j * 512:(j + 1) * 512],
                        lhsT=w_sb["wq"][cc][:, m * 128:(m + 1) * 128],
                        rhs=t2T[cc][:, j * 512:(j + 1) * 512],
                        start=(cc == 0), stop=(cc == 1))
                nc.vector.tensor_copy(qT[m][:, j * 512:(j + 1) * 512],
                                      ps[:, j * 512:(j + 1) * 512])

        def emit_v(batch):
            ps = spool.tile([128, 1024], FP32, name="s", tag="s")
            for i in range(4):
                kc = batch * 4 + i
                for cc in range(2):
                    nc.tensor.matmul(
                        ps[:, i * 256:(i + 1) * 256],
                        lhsT=t1T[cc][:, kc * 128:(kc + 1) * 128],
                        rhs=w_sb["wv"][cc][:],
                        start=(cc == 0), stop=(cc == 1))
            src = ps[:].rearrange("p (k h c) -> p k h c", k=4, h=4)
            dst = v_sb[:, batch * 1280:(batch + 1) * 1280] \
                .rearrange("p (k h c) -> p k h c", k=4, h=4, c=80)[:, :, :, 1:65]
            nc.vector.tensor_copy(dst, src)

        def emit_v2(kcp):
            ps = spool.tile([128, 1024], FP32, name="s", tag="s")
            for t in range(2):
                kc = kcp * 2 + t
                for cc in range(2):
                    nc.tensor.matmul(
                        ps[:, t * 256:(t + 1) * 256],
                        lhsT=t1T[cc][:, kc * 128:(kc + 1) * 128],
                        rhs=w_sb["wv"][cc][:],
                        start=(cc == 0), stop=(cc == 1))
            src = ps[:, 0:512].rearrange("p (k h c) -> p k h c", k=2, h=4)
            dst = v_sb[:, kcp * 640:(kcp + 1) * 640] \
                .rearrange("p (k h c) -> p k h c", k=2, h=4, c=80)[:, :, :, 1:65]
            nc.vector.tensor_copy(dst, src)

        def pes_slot(kcp, j):
            off = (kcp * 2 + j) * 2048
            return pes_sb[:, off:off + 2048]

        def emit_s_exp(m, kc, j):
            s = spool.tile([128, 1024], FP32, name="s", tag="s")
            for hh in range(2):
                nc.tensor.matmul(
                    s[:, hh * 512:(hh + 1) * 512],
                    lhsT=kT[m][hh * 64:(hh + 1) * 64,
                               kc * 128:(kc + 1) * 128],
                    rhs=qT[m][hh * 64:(hh + 1) * 64, j * 512:(j + 1) * 512],
                    start=True, stop=True)
            kcp, t = kc // 2, kc & 1
            dst = pes_slot(kcp, j)
            if dve_rule(m * 8 + kcp, j):
                du8 = dst.bitcast(U8)[:, t * 1024:(t + 1) * 1024]
                nc.vector.tensor_scalar(du8, s[:], A5, B5, ALU.mult, ALU.add)
            else:
                nc.scalar.activation(dst[:, t * 1024:(t + 1) * 1024], s[:],
                                     AF.Exp, scale=SCALE)

        def emit_xo(m, kcp, j, xo_ps):
            rhs = pes_slot(kcp, j).rearrange("p (t q) -> p t q", t=2)
            for hh in range(2):
                h = 2 * m + hh
                lhsT = v_sb[:, kcp * 640:(kcp + 1) * 640] \
                    .rearrange("p (t h c) -> p t h c", t=2, h=4)[:, :, h, 0:65]
                nc.tensor.matmul(
                    xo_ps[hh][0:65, :],
                    lhsT=lhsT,
                    rhs=rhs[:, :, hh * 512:(hh + 1) * 512],
                    start=(kcp == 0), stop=(kcp == 7),
                    perf_mode=DR)

        def norm(m, j, xo_ps):
            for hh in range(2):
                recip = npool.tile([1, 512], FP32, name=f"recip{hh}",
                                   tag=f"recip{hh}")
                nc.vector.reciprocal_approx_fast(recip[:], xo_ps[hh][0:1, :])
                bc = npool.tile([65, 512], FP32, name=f"bc{hh}",
                                tag=f"bc{hh}")
                nc.gpsimd.partition_broadcast(bc[:], recip[:])
                nc.vector.tensor_mul(
                    xon[2 * m + hh][:, j * 512:(j + 1) * 512],
                    xo_ps[hh][0:65, :], bc[:])

        emit_kT(0, 0, halves=(0,))
        emit_qT(0, halves=(0,))

        if FINE_P1:
            phase1_work = [
                lambda: emit_v2(0),
                lambda: emit_kT(0, 0, halves=(1,)),
                lambda: emit_v2(1),
                lambda: emit_qT(0, halves=(1,)),
                lambda: emit_v2(2),
                lambda: emit_v2(3),
                lambda: emit_kT(0, 1, halves=(0,)),
                lambda: emit_kT(0, 1, halves=(1,)),
                lambda: emit_v2(4),
                lambda: emit_v2(5),
                lambda: emit_kT(1, 0, halves=(0,)),
                lambda: emit_v2(6),
                lambda: emit_kT(1, 0, halves=(1,)),
                lambda: emit_v2(7),
                lambda: emit_kT(1, 1),
                lambda: emit_qT(1)]
        else:
            phase1_work = [lambda: emit_v(0),
                           lambda: emit_kT(0, 0, halves=(1,)),
                           lambda: emit_qT(0, halves=(1,)),
                           lambda: emit_v(1),
                           lambda: emit_kT(0, 1),
                           lambda: emit_v(2), lambda: emit_v(3),
                           lambda: emit_kT(1, 0), lambda: emit_kT(1, 1),
                           lambda: emit_qT(1)]

        phase2_work = []
        if P1_SPLIT:
            phase1_work = [lambda: emit_v(0),
                           lambda: emit_kT(0, 0, halves=(1,)),
                           lambda: emit_qT(0, halves=(1,)),
                           lambda: emit_v(1),
                           lambda: emit_kT(0, 1),
                           lambda: emit_v(2), lambda: emit_v(3)]
            phase2_work = [lambda: emit_kT(1, 0), lambda: emit_kT(1, 1),
                           lambda: emit_qT(1)]

        pending_norm = []
        for m in range(2):
            for jp in range(2):
                xo_ps = [xopool.tile([65, 512], FP32, name=f"xo{hh}",
                                     tag=f"xo{hh}") for hh in range(2)]
                for kc in range(16):
                    emit_s_exp(m, kc, jp)
                    if m == 0 and jp == 0 and kc >= POP_FROM and phase1_work:
                        phase1_work.pop(0)()
                    if m == 0 and jp == 1 and (kc & 1) and phase2_work:
                        phase2_work.pop(0)()
                    if kc & 1:
                        emit_xo(m, kc // 2, jp, xo_ps)
                    if kc == NORM_KC and pending_norm:
                        norm(*pending_norm.pop())
                pending_norm.append((m, jp, xo_ps))
            if m == 0:
                for cc in range(2):
                    nc.sync.dma_start(out=w_sb["wc"][cc][:],
                                      in_=wc_d[cc * 128:(cc + 1) * 128, :])
                for h in range(4):
                    nc.sync.dma_start(out=wph[h][:],
                                      in_=wph_d[h * 65:(h + 1) * 65, :])

        while pending_norm:
            norm(*pending_norm.pop(0))

        ot = []

        def emit_projA():
            for ch in range(2):
                o = spool.tile([128, 1024], FP32, name="s", tag="s")
                ot.append(o)
                for j in range(2):
                    for cc in range(2):
                        nc.tensor.matmul(
                            o[:, j * 512:(j + 1) * 512],
                            lhsT=w_sb["wc"][cc][:, ch * 128:(ch + 1) * 128],
                            rhs=t2T[cc][:, j * 512:(j + 1) * 512],
                            start=(cc == 0), stop=False)
                    for h in range(2):
                        nc.tensor.matmul(
                            o[:, j * 512:(j + 1) * 512],
                            lhsT=wph[h][:, ch * 128:(ch + 1) * 128],
                            rhs=xon[h][:, j * 512:(j + 1) * 512],
                            start=False, stop=False)

        emit_projA()

        for ch in range(2):
            o = ot[ch]
            o_sb = osb.tile([128, 1024], BF16, name="o", tag="o")
            for j in range(2):
                for h in range(2, 4):
                    nc.tensor.matmul(
                        o[:, j * 512:(j + 1) * 512],
                        lhsT=wph[h][:, ch * 128:(ch + 1) * 128],
                        rhs=xon[h][:, j * 512:(j + 1) * 512],
                        start=False, stop=(h == 3))
                if j == 1 or EVAC_BOTH_SCALAR:
                    nc.scalar.copy(o_sb[:, j * 512:(j + 1) * 512],
                                   o[:, j * 512:(j + 1) * 512])
                else:
                    nc.vector.tensor_copy(o_sb[:, j * 512:(j + 1) * 512],
                                          o[:, j * 512:(j + 1) * 512])
                nc.sync.dma_start(
                    out=out_d[ch * 128:(ch + 1) * 128, j * 512:(j + 1) * 512],
                    in_=o_sb[:, j * 512:(j + 1) * 512])

    nc.finalize()
    return nc


def _get_nc():
    if "nc" not in _CACHE:
        _CACHE["nc"] = build_nc()
    return _CACHE["nc"]


def make_in_maps(t2_grad, t1, Wq, Wkv, Wproj, bproj):
    bf16 = ml_dtypes.bfloat16
    t2 = np.asarray(t2_grad, dtype=np.float32)
    t1 = np.asarray(t1, dtype=np.float32)
    wq = np.ascontiguousarray(Wq, dtype=np.float32)
    wk = np.ascontiguousarray(Wkv[:, :C]).astype(bf16)
    wv = np.ascontiguousarray(Wkv[:, C:]).astype(bf16)
    wp64 = np.asarray(Wproj, dtype=np.float64)
    wcomb = ((np.eye(C) + np.asarray(Wq, np.float64)) @ wp64).astype(bf16)
    wph = np.zeros((4 * 65, C), dtype=np.float32)
    for h in range(4):
        wph[h * 65 + 1:(h + 1) * 65] = np.asarray(Wproj[h * 64:(h + 1) * 64],
                                                  np.float32)
    wph[0] = np.asarray(bproj, np.float32)
    wph = wph.astype(bf16)
    wq_b = wq.astype(bf16)

    in_maps = []
    for c in range(NCORES):
        b, qh = c // 2, c % 2
        in_maps.append({
            "t1T": np.ascontiguousarray(t1[b].T).astype(bf16),
            "t2T": np.ascontiguousarray(t2[b].T[:, qh * Q:(qh + 1) * Q]).astype(bf16),
            "wq": wq_b, "wk": wk, "wv": wv, "wc": wcomb, "wph": wph,
        })
    return in_maps


def kernel(t2_grad, t1, Wq, Wkv, Wproj, bproj, gamma, _trace=False,
           _use_fp32r=None):
    gamma = np.asarray(gamma)
    if float(np.abs(gamma).max()) != 0.0:
        return _host_reference(t2_grad, t1, Wq, Wkv, Wproj, bproj, gamma)

    nc = _get_nc()
    in_maps = make_in_maps(t2_grad, t1, Wq, Wkv, Wproj, bproj)
    res = run_bass_kernel_spmd(nc, in_maps, list(range(NCORES)), trace=_trace)
    out = np.empty((B, N, C), dtype=np.float32)
    for c in range(NCORES):
        b, qh = c // 2, c % 2
        out[b, qh * Q:(qh + 1) * Q, :] = \
            np.asarray(res.results[c]["out"]).astype(np.float32).T
    if _trace:
        _CACHE["last_result"] = res
    return out


def _host_reference(t2_grad, t1, Wq, Wkv, Wproj, bproj, gamma):
    t2 = np.asarray(t2_grad, dtype=np.float64)
    t1 = np.asarray(t1, dtype=np.float64)
    Wq = np.asarray(Wq, dtype=np.float64)
    Wkv = np.asarray(Wkv, dtype=np.float64)
    Wproj = np.asarray(Wproj, dtype=np.float64)
    bproj = np.asarray(bproj, dtype=np.float64)
    g = float(np.asarray(gamma).reshape(-1)[0])
    q = (t2 @ Wq).reshape(B, N, H, D).transpose(0, 2, 1, 3)
    kv = (t1 @ Wkv).reshape(B, N, 2, H, D).transpose(2, 0, 3, 1, 4)
    k, v = kv[0], kv[1]
    s = np.einsum('bhnd,bhmd->bhnm', q, k) * SCALE
    s = s - s.max(axis=-1, keepdims=True)
    p = np.exp(s)
    p /= p.sum(axis=-1, keepdims=True)
    x = np.einsum('bhnm,bhmd->bhnd', p, v)
    xp = x.transpose(0, 3, 1, 2).reshape(B, D, H * N)
    energy = xp @ xp.transpose(0, 2, 1)
    energy = energy - energy.max(axis=-1, keepdims=True)
    att = np.exp(energy)
    att /= att.sum(axis=-1, keepdims=True)
    lam_out = (att @ xp).reshape(B, D, H, N)
    lam_out = g * lam_out + xp.reshape(B, D, H, N)
    x = lam_out.transpose(0, 2, 3, 1)
    xo = x.transpose(0, 2, 1, 3).reshape(B, N, C) \
        + q.transpose(0, 2, 1, 3).reshape(B, N, C)
    return ((t2 + xo) @ Wproj + bproj).astype(np.float32)


# revision 36
# speedup vs baseline: 1.1780x; 1.1780x over previous
from contextlib import ExitStack

import numpy as np
import ml_dtypes

import concourse.bass as bass
import concourse.mybir as mybir
import concourse.tile as tile
from concourse import bacc
from concourse.bass_utils import run_bass_kernel_spmd

B, N, C, H, D = 4, 2048, 256, 4, 64
NCORES = 8
Q = 1024
SCALE = float(D) ** -0.5
FP32 = mybir.dt.float32
BF16 = mybir.dt.bfloat16
FP8E4 = mybir.dt.float8e4
FP8E5 = mybir.dt.float8e5
U8 = mybir.dt.uint8
AF = mybir.ActivationFunctionType
DR = mybir.MatmulPerfMode.DoubleRow
ALU = mybir.AluOpType

A5 = 4.0 * np.log2(np.e) * SCALE
B5 = 60.0


def dve_tile(g, j):
    return ((2 * g + j) % 3 == 1 or g % 8 == 6) and g % 8 < 7


_CACHE = {}
FINE_P1 = False
P1_SPLIT = False
POP_FROM = 1
NORM_KC = 3
EVAC_BOTH_SCALAR = False
T1FP8 = True


def build_nc(dve_rule=None, warmup_n=10):
    if dve_rule is None:
        dve_rule = dve_tile
    nc = bacc.Bacc("TRN2", target_bir_lowering=False, debug=False,
                   num_devices=NCORES)

    t1T_d = nc.dram_tensor("t1T", [C, N],
                           FP8E4 if T1FP8 else BF16,
                           kind="ExternalInput")
    t2T_d = nc.dram_tensor("t2T", [C, Q], BF16, kind="ExternalInput")
    wq_d = nc.dram_tensor("wq", [C, C], BF16, kind="ExternalInput")
    wk_d = nc.dram_tensor("wk", [C, C], BF16, kind="ExternalInput")
    wv_d = nc.dram_tensor("wv", [C, C], BF16, kind="ExternalInput")
    wc_d = nc.dram_tensor("wc", [C, C], BF16, kind="ExternalInput")
    wph_d = nc.dram_tensor("wph", [4 * 65, C], BF16, kind="ExternalInput")
    out_d = nc.dram_tensor("out", [C, Q], BF16, kind="ExternalOutput")

    with tile.TileContext(nc) as tc, ExitStack() as ctx:
        const = ctx.enter_context(tc.tile_pool(name="const", bufs=1))
        acts = ctx.enter_context(tc.tile_pool(name="acts", bufs=1))

        w_sb = {n: [const.tile([128, C], BF16, name=f"{n}{cc}", tag=f"{n}{cc}")
                    for cc in range(2)]
                for n in ("wk", "wq", "wv", "wc")}
        wph = [const.tile([65, C], BF16, name=f"wph{h}", tag=f"wph{h}")
               for h in range(4)]
        t1T = [acts.tile([128, N], FP8E4 if T1FP8 else BF16,
                         name=f"t1T{cc}", tag=f"t1T{cc}")
               for cc in range(2)]
        t2T = [acts.tile([128, Q], BF16, name=f"t2T{cc}", tag=f"t2T{cc}")
               for cc in range(2)]

        kT = [acts.tile([128, N], BF16, name=f"kT{m}", tag=f"kT{m}")
              for m in range(2)]
        qT = [acts.tile([128, Q], BF16, name=f"qT{m}", tag=f"qT{m}")
              for m in range(2)]
        wmat = const.tile([128, 512], BF16, name="wmat", tag="wmat")
        nc.gpsimd.memset(wmat[:], 0.0)
        v_sb = acts.tile([128, 8 * 640], FP8E4, name="v", tag="v")
        for kcp in range(8):
            nc.gpsimd.memset(
                v_sb[:, kcp * 640:(kcp + 1) * 640]
                .rearrange("p (t h c) -> p t h c", t=2, h=4)[:, :, :, 0:1],
                1.0)
        xon = [acts.tile([65, Q], BF16, name=f"xon{h}", tag=f"xon{h}")
               for h in range(4)]

        pes_sb = acts.tile([128, 16 * 2048], FP8E5, name="pes", tag="pes")
        npool = ctx.enter_context(tc.tile_pool(name="norm", bufs=2))
        osb = ctx.enter_context(tc.tile_pool(name="osb", bufs=2))

        spool = ctx.enter_context(
            tc.tile_pool(name="spsum", bufs=3, space="PSUM"))
        xopool = ctx.enter_context(
            tc.tile_pool(name="xopsum", bufs=1, space="PSUM"))

        qs = (nc.sync, nc.scalar)
        for cc in range(2):
            qs[cc].dma_start(out=w_sb["wk"][cc][:],
                             in_=wk_d[cc * 128:(cc + 1) * 128, :])
        for cc in range(2):
            qs[cc].dma_start(out=t1T[cc][:, 0:512],
                             in_=t1T_d[cc * 128:(cc + 1) * 128, 0:512])
        for cc in range(2):
            qs[cc].dma_start(out=w_sb["wq"][cc][:],
                             in_=wq_d[cc * 128:(cc + 1) * 128, :])
        for cc in range(2):
            qs[cc].dma_start(out=t2T[cc][:, 0:512],
                             in_=t2T_d[cc * 128:(cc + 1) * 128, 0:512])
        for cc in range(2):
            qs[cc].dma_start(out=t1T[cc][:, 512:1024],
                             in_=t1T_d[cc * 128:(cc + 1) * 128, 512:1024])
        for cc in range(2):
            qs[cc].dma_start(out=w_sb["wv"][cc][:],
                             in_=wv_d[cc * 128:(cc + 1) * 128, :])
        for cc in range(2):
            qs[cc].dma_start(out=t2T[cc][:, 512:1024],
                             in_=t2T_d[cc * 128:(cc + 1) * 128, 512:1024])
        for cc in range(2):
            qs[cc].dma_start(out=t1T[cc][:, 1024:2048],
                             in_=t1T_d[cc * 128:(cc + 1) * 128, 1024:2048])

        warm = const.tile([1, 16], FP32, name="warm", tag="warm")
        nc.gpsimd.memset(warm[:], 0.0)
        nc.scalar.activation(warm[:], warm[:], AF.Exp)
        wxo = xopool.tile([65, 512], FP32, name="xo0", tag="xo0")
        for _ in range(warmup_n):
            nc.tensor.matmul(wxo[0:65, :], lhsT=wmat[:, 0:65], rhs=wmat[:],
                             start=True, stop=True, skip_group_check=True)

        def emit_kT(m, blk, halves=(0, 1)):
            ps = spool.tile([128, 1024], FP32, name="s", tag="s")
            for j in halves:
                for cc in range(2):
                    nc.tensor.matmul(
                        ps[:, j * 512:(j + 1) * 512],
                        lhsT=w_sb["wk"][cc][:, m * 128:(m + 1) * 128],
                        rhs=t1T[cc][:, blk * 1024 + j * 512:
                                    blk * 1024 + (j + 1) * 512],
                        start=(cc == 0), stop=(cc == 1))
                nc.vector.tensor_copy(
                    kT[m][:, blk * 1024 + j * 512:blk * 1024 + (j + 1) * 512],
                    ps[:, j * 512:(j + 1) * 512])

        def emit_qT(m, halves=(0, 1)):
            ps = spool.tile([128, 1024], FP32, name="s", tag="s")
            for j in halves:
                for cc in range(2):
                    nc.tensor.matmul(
                        ps[:, j * 512:(j + 1) * 512],
                        lhsT=w_sb["wq"][cc][:, m * 128:(m + 1) * 128],
                        rhs=t2T[cc][:, j * 512:(j + 1) * 512],
                        start=(cc == 0), stop=(cc == 1))
                nc.vector.tensor_copy(qT[m][:, j * 512:(j + 1) * 512],
                                      ps[:, j * 512:(j + 1) * 512])

        def emit_v(batch):
            ps = spool.tile([128, 1024], FP32, name="s", tag="s")
            for i in range(4):
                kc = batch * 4 + i
                for cc in range(2):
                    nc.tensor.matmul(
                        ps[:, i * 256:(i + 1) * 256],
                        lhsT=t1T[cc][:, kc * 128:(kc + 1) * 128],
                        rhs=w_sb["wv"][cc][:],
                        start=(cc == 0), stop=(cc == 1))
            src = ps[:].rearrange("p (k h c) -> p k h c", k=4, h=4)
            dst = v_sb[:, batch * 1280:(batch + 1) * 1280] \
                .rearrange("p (k h c) -> p k h c", k=4, h=4, c=80)[:, :, :, 1:65]
            nc.vector.tensor_copy(dst, src)

        def emit_v2(kcp):
            ps = spool.tile([128, 1024], FP32, name="s", tag="s")
            for t in range(2):
                kc = kcp * 2 + t
                for cc in range(2):
                    nc.tensor.matmul(
                        ps[:, t * 256:(t + 1) * 256],
                        lhsT=t1T[cc][:, kc * 128:(kc + 1) * 128],
                        rhs=w_sb["wv"][cc][:],
                        start=(cc == 0), stop=(cc == 1))
            src = ps[:, 0:512].rearrange("p (k h c) -> p k h c", k=2, h=4)
            dst = v_sb[:, kcp * 640:(kcp + 1) * 640] \
                .rearrange("p (k h c) -> p k h c", k=2, h=4, c=80)[:, :, :, 1:65]
            nc.vector.tensor_copy(dst, src)

        def pes_slot(kcp, j):
            off = (kcp * 2 + j) * 2048
            return pes_sb[:, off:off + 2048]

        def emit_s_exp(m, kc, j):
            s = spool.tile([128, 1024], FP32, name="s", tag="s")
            for hh in range(2):
                nc.tensor.matmul(
                    s[:, hh * 512:(hh + 1) * 512],
                    lhsT=kT[m][hh * 64:(hh + 1) * 64,
                               kc * 128:(kc + 1) * 128],
                    rhs=qT[m][hh * 64:(hh + 1) * 64, j * 512:(j + 1) * 512],
                    start=True, stop=True)
            kcp, t = kc // 2, kc & 1
            dst = pes_slot(kcp, j)
            if dve_rule(m * 8 + kcp, j):
                du8 = dst.bitcast(U8)[:, t * 1024:(t + 1) * 1024]
                nc.vector.tensor_scalar(du8, s[:], A5, B5, ALU.mult, ALU.add)
            else:
                nc.scalar.activation(dst[:, t * 1024:(t + 1) * 1024], s[:],
                                     AF.Exp, scale=SCALE)

        def emit_xo(m, kcp, j, xo_ps):
            rhs = pes_slot(kcp, j).rearrange("p (t q) -> p t q", t=2)
            for hh in range(2):
                h = 2 * m + hh
                lhsT = v_sb[:, kcp * 640:(kcp + 1) * 640] \
                    .rearrange("p (t h c) -> p t h c", t=2, h=4)[:, :, h, 0:65]
                nc.tensor.matmul(
                    xo_ps[hh][0:65, :],
                    lhsT=lhsT,
                    rhs=rhs[:, :, hh * 512:(hh + 1) * 512],
                    start=(kcp == 0), stop=(kcp == 7),
                    perf_mode=DR)

        def norm(m, j, xo_ps):
            for hh in range(2):
                recip = npool.tile([1, 512], FP32, name=f"recip{hh}",
                                   tag=f"recip{hh}")
                nc.vector.reciprocal_approx_fast(recip[:], xo_ps[hh][0:1, :])
                bc = npool.tile([65, 512], FP32, name=f"bc{hh}",
                                tag=f"bc{hh}")
                nc.gpsimd.partition_broadcast(bc[:], recip[:])
                nc.vector.tensor_mul(
                    xon[2 * m + hh][:, j * 512:(j + 1) * 512],
                    xo_ps[hh][0:65, :], bc[:])

        emit_kT(0, 0, halves=(0,))
        emit_qT(0, halves=(0,))

        if FINE_P1:
            phase1_work = [
                lambda: emit_v2(0),
                lambda: emit_kT(0, 0, halves=(1,)),
                lambda: emit_v2(1),
                lambda: emit_qT(0, halves=(1,)),
                lambda: emit_v2(2),
                lambda: emit_v2(3),
                lambda: emit_kT(0, 1, halves=(0,)),
                lambda: emit_kT(0, 1, halves=(1,)),
                lambda: emit_v2(4),
                lambda: emit_v2(5),
                lambda: emit_kT(1, 0, halves=(0,)),
                lambda: emit_v2(6),
                lambda: emit_kT(1, 0, halves=(1,)),
                lambda: emit_v2(7),
                lambda: emit_kT(1, 1),
                lambda: emit_qT(1)]
        else:
            phase1_work = [lambda: emit_v(0),
                           lambda: emit_kT(0, 0, halves=(1,)),
                           lambda: emit_qT(0, halves=(1,)),
                           lambda: emit_v(1),
                           lambda: emit_kT(0, 1),
                           lambda: emit_v(2), lambda: emit_v(3),
                           lambda: emit_kT(1, 0), lambda: emit_kT(1, 1),
                           lambda: emit_qT(1)]

        phase2_work = []
        if P1_SPLIT:
            phase1_work = [lambda: emit_v(0),
                           lambda: emit_kT(0, 0, halves=(1,)),
                           lambda: emit_qT(0, halves=(1,)),
                           lambda: emit_v(1),
                           lambda: emit_kT(0, 1),
                           lambda: emit_v(2), lambda: emit_v(3)]
            phase2_work = [lambda: emit_kT(1, 0), lambda: emit_kT(1, 1),
                           lambda: emit_qT(1)]

        pending_norm = []
        for m in range(2):
            for jp in range(2):
                xo_ps = [xopool.tile([65, 512], FP32, name=f"xo{hh}",
                                     tag=f"xo{hh}") for hh in range(2)]
                for kc in range(16):
                    emit_s_exp(m, kc, jp)
                    if m == 0 and jp == 0 and kc >= POP_FROM and phase1_work:
                        phase1_work.pop(0)()
                    if m == 0 and jp == 1 and (kc & 1) and phase2_work:
                        phase2_work.pop(0)()
                    if kc & 1:
                        emit_xo(m, kc // 2, jp, xo_ps)
                    if kc == NORM_KC and pending_norm:
                        norm(*pending_norm.pop())
                pending_norm.append((m, jp, xo_ps))
            if m == 0:
                for cc in range(2):
                    nc.sync.dma_start(out=w_sb["wc"][cc][:],
                                      in_=wc_d[cc * 128:(cc + 1) * 128, :])
                for h in range(4):
                    nc.sync.dma_start(out=wph[h][:],
                                      in_=wph_d[h * 65:(h + 1) * 65, :])

        while pending_norm:
            norm(*pending_norm.pop(0))

        ot = []

        def emit_projA():
            for ch in range(2):
                o = spool.tile([128, 1024], FP32, name="s", tag="s")
                ot.append(o)
                for j in range(2):
                    for cc in range(2):
                        nc.tensor.matmul(
                            o[:, j * 512:(j + 1) * 512],
                            lhsT=w_sb["wc"][cc][:, ch * 128:(ch + 1) * 128],
                            rhs=t2T[cc][:, j * 512:(j + 1) * 512],
                            start=(cc == 0), stop=False)
                    for h in range(2):
                        nc.tensor.matmul(
                            o[:, j * 512:(j + 1) * 512],
                            lhsT=wph[h][:, ch * 128:(ch + 1) * 128],
                            rhs=xon[h][:, j * 512:(j + 1) * 512],
                            start=False, stop=False)

        emit_projA()

        for ch in range(2):
            o = ot[ch]
            o_sb = osb.tile([128, 1024], BF16, name="o", tag="o")
            for j in range(2):
                for h in range(2, 4):
                    nc.tensor.matmul(
                        o[:, j * 512:(j + 1) * 512],
                        lhsT=wph[h][:, ch * 128:(ch + 1) * 128],
                        rhs=xon[h][:, j * 512:(j + 1) * 512],
                        start=False, stop=(h == 3))
                if j == 1 or EVAC_BOTH_SCALAR:
                    nc.scalar.copy(o_sb[:, j * 512:(j + 1) * 512],
                                   o[:, j * 512:(j + 1) * 512])
                else:
                    nc.vector.tensor_copy(o_sb[:, j * 512:(j + 1) * 512],
                                          o[:, j * 512:(j + 1) * 512])
                nc.sync.dma_start(
                    out=out_d[ch * 128:(ch + 1) * 128, j * 512:(j + 1) * 512],
                    in_=o_sb[:, j * 512:(j + 1) * 512])

    nc.finalize()
    return nc


def _get_nc():
    if "nc" not in _CACHE:
        _CACHE["nc"] = build_nc()
    return _CACHE["nc"]


def make_in_maps(t2_grad, t1, Wq, Wkv, Wproj, bproj):
    bf16 = ml_dtypes.bfloat16
    t2 = np.asarray(t2_grad, dtype=np.float32)
    t1 = np.asarray(t1, dtype=np.float32)
    wq = np.ascontiguousarray(Wq, dtype=np.float32)
    wk = np.ascontiguousarray(Wkv[:, :C]).astype(bf16)
    wv = np.ascontiguousarray(Wkv[:, C:]).astype(bf16)
    wp64 = np.asarray(Wproj, dtype=np.float64)
    wcomb = ((np.eye(C) + np.asarray(Wq, np.float64)) @ wp64).astype(bf16)
    wph = np.zeros((4 * 65, C), dtype=np.float32)
    for h in range(4):
        wph[h * 65 + 1:(h + 1) * 65] = np.asarray(Wproj[h * 64:(h + 1) * 64],
                                                  np.float32)
    wph[0] = np.asarray(bproj, np.float32)
    wph = wph.astype(bf16)
    wq_b = wq.astype(bf16)

    t1dt = ml_dtypes.float8_e4m3 if T1FP8 else bf16
    in_maps = []
    for c in range(NCORES):
        b, qh = c // 2, c % 2
        in_maps.append({
            "t1T": np.ascontiguousarray(t1[b].T).astype(t1dt),
            "t2T": np.ascontiguousarray(t2[b].T[:, qh * Q:(qh + 1) * Q]).astype(bf16),
            "wq": wq_b, "wk": wk, "wv": wv, "wc": wcomb, "wph": wph,
        })
    return in_maps


def kernel(t2_grad, t1, Wq, Wkv, Wproj, bproj, gamma, _trace=False,
           _use_fp32r=None):
    gamma = np.asarray(gamma)
    if float(np.abs(gamma).max()) != 0.0:
        return _host_reference(t2_grad, t1, Wq, Wkv, Wproj, bproj, gamma)

    nc = _get_nc()
    in_maps = make_in_maps(t2_grad, t1, Wq, Wkv, Wproj, bproj)
    res = run_bass_kernel_spmd(nc, in_maps, list(range(NCORES)), trace=_trace)
    out = np.empty((B, N, C), dtype=np.float32)
    for c in range(NCORES):
        b, qh = c // 2, c % 2
        out[b, qh * Q:(qh + 1) * Q, :] = \
            np.asarray(res.results[c]["out"]).astype(np.float32).T
    if _trace:
        _CACHE["last_result"] = res
    return out


def _host_reference(t2_grad, t1, Wq, Wkv, Wproj, bproj, gamma):
    t2 = np.asarray(t2_grad, dtype=np.float64)
    t1 = np.asarray(t1, dtype=np.float64)
    Wq = np.asarray(Wq, dtype=np.float64)
    Wkv = np.asarray(Wkv, dtype=np.float64)
    Wproj = np.asarray(Wproj, dtype=np.float64)
    bproj = np.asarray(bproj, dtype=np.float64)
    g = float(np.asarray(gamma).reshape(-1)[0])
    q = (t2 @ Wq).reshape(B, N, H, D).transpose(0, 2, 1, 3)
    kv = (t1 @ Wkv).reshape(B, N, 2, H, D).transpose(2, 0, 3, 1, 4)
    k, v = kv[0], kv[1]
    s = np.einsum('bhnd,bhmd->bhnm', q, k) * SCALE
    s = s - s.max(axis=-1, keepdims=True)
    p = np.exp(s)
    p /= p.sum(axis=-1, keepdims=True)
    x = np.einsum('bhnm,bhmd->bhnd', p, v)
    xp = x.transpose(0, 3, 1, 2).reshape(B, D, H * N)
    energy = xp @ xp.transpose(0, 2, 1)
    energy = energy - energy.max(axis=-1, keepdims=True)
    att = np.exp(energy)
    att /= att.sum(axis=-1, keepdims=True)
    lam_out = (att @ xp).reshape(B, D, H, N)
    lam_out = g * lam_out + xp.reshape(B, D, H, N)
    x = lam_out.transpose(0, 2, 3, 1)
    xo = x.transpose(0, 2, 1, 3).reshape(B, N, C) \
        + q.transpose(0, 2, 1, 3).reshape(B, N, C)
    return ((t2 + xo) @ Wproj + bproj).astype(np.float32)
